# revision 17
# baseline (speedup 1.0000x reference)
"""Trainium2 Bass kernel for nn_MultiHeadAttention_42640435315371.

Data-parallel over 8 NeuronCores: each core handles 2048 of the 16384
(n*t) tokens; the four d_model x d_model weights are replicated (shipped
bf16, pre-transposed/permuted on host).

Math notes (matching reference.py exactly):
  - energy_t = Qh_t^T Kh_t / 32 per token (token-local "attention");
    the 1/32 scale and the mask are folded into K as K * mask/32, so a
    masked token yields an all-zero energy matrix -> softmax = uniform
    1/64, identical to softmax of a constant -1e20 row.
  - energies are tiny (|E| < ~1), so exp() needs no max-subtraction.
  - concat order is (d_head, head); Wo's columns are permuted on host so
    the device can emit rows k = h*64 + i.

v2: inputs shipped bf16 (halves input HBM); all shuffle staging through
DRAM replaced by direct SBUF->SBUF DMAs with composed access patterns;
head outputs staged in a per-tc4 [64, 16, 128] tile so the concat
reassembly DMA uses 512B runs.
"""

import os
import os
import numpy as np

import concourse.bass as bass
import concourse.mybir as mybir
from concourse import bacc
from concourse.tile import TileContext
from concourse.bass_utils import run_bass_kernel_spmd

F32 = mybir.dt.float32
BF16 = mybir.dt.bfloat16

K_STAGE = int(os.environ.get("K_STAGE", "99"))
K_STAGE = int(os.environ.get("K_STAGE", "99"))
N_CORES = 8
N, T, D, H, DH = 4, 4096, 1024, 16, 64
TOK = (N * T) // N_CORES  # 2048 tokens per core
MT = 512                  # megatile tokens
NMT = TOK // MT


def build_nc():
    nc = bacc.Bacc("TRN2", target_bir_lowering=False, debug=False,
                   num_devices=N_CORES)
    xq = nc.declare_dram_parameter("xq", [D, TOK], BF16, isOutput=False)
    xk = nc.declare_dram_parameter("xk", [D, TOK], BF16, isOutput=False)
    xv = nc.declare_dram_parameter("xv", [D, TOK], BF16, isOutput=False)
    wq = nc.declare_dram_parameter("wq", [D, D], BF16, isOutput=False)
    wk = nc.declare_dram_parameter("wk", [D, D], BF16, isOutput=False)
    wv = nc.declare_dram_parameter("wv", [D, D], BF16, isOutput=False)
    wo = nc.declare_dram_parameter("wo", [D, D], BF16, isOutput=False)
    m32 = nc.declare_dram_parameter("m32", [128, TOK // 128], F32, isOutput=False)
    out = nc.declare_dram_parameter("out", [D, TOK], F32, isOutput=True)

    from contextlib import ExitStack
    with TileContext(nc) as tc, ExitStack() as ctx:
        const = ctx.enter_context(tc.tile_pool(name="const", bufs=1))
        p_xb = ctx.enter_context(tc.tile_pool(name="xb", bufs=8))
        p_maj = ctx.enter_context(tc.tile_pool(name="maj", bufs=2))
        p_cc = ctx.enter_context(tc.tile_pool(name="cc", bufs=8))
        p_exp = ctx.enter_context(tc.tile_pool(name="expp", bufs=3))
        p_hd = ctx.enter_context(tc.tile_pool(name="hd", bufs=2))
        p_rcp = ctx.enter_context(tc.tile_pool(name="rcp", bufs=1))
        p_outT = ctx.enter_context(tc.tile_pool(name="outT", bufs=2))
        ps_proj = ctx.enter_context(tc.tile_pool(name="psp", bufs=2, space="PSUM"))
        ps_E = ctx.enter_context(tc.tile_pool(name="psE", bufs=3, space="PSUM"))
        ps_2 = ctx.enter_context(tc.tile_pool(name="ps2", bufs=2, space="PSUM"))
        p_vt = ctx.enter_context(tc.tile_pool(name="vt", bufs=16))
        p_stg = ctx.enter_context(tc.tile_pool(name="stg", bufs=2, space="DRAM"))

        # ---- static tiles ----
        def load_w(name, dram):
            tiles = []
            for i in range(8):
                t = const.tile([128, D], BF16, tag=f"{name}{i}")
                nc.sync.dma_start(out=t[:], in_=dram[i * 128:(i + 1) * 128, :])
                tiles.append(t)
            return tiles

        wq_sb, wk_sb, wv_sb, wo_sb = (load_w(n, d) for n, d in
                                      (("wq", wq), ("wk", wk), ("wv", wv), ("wo", wo)))
        m32_sb = const.tile([128, TOK // 128], F32, tag="m32")
        nc.sync.dma_start(out=m32_sb[:], in_=m32[:])

        # Packed per-tc4 tiles (2x ping-pong; zeros/ones written once).
        # stqT [32=(b2,h16), 64g*64i]: stqT[(b,h),(g,i)] = Q[2g+b, h*64+i]
        # bdkT [32=(b2,h16), 64g*2y*64j]: nonzero strip y==b holds
        #   K[2g+b, h*64+j] (block-diagonal in (b,y)).
        # bdvT [128=(b2,j64), 64g*34c]: cols c=b'*16+h hold V[2g+b, j, h]
        #   for b'==b; cols 32+b' hold ones on partition-half b'.
        stqT_pp, bdkT_pp, bdvT_pp = [], [], []
        for i in range(2):
            t = const.tile([32, 64 * 64], BF16, tag=f"stqT{i}")
            stqT_pp.append(t)
            t = const.tile([32, 64 * 128], BF16, tag=f"bdkT{i}")
            nc.vector.memset(t[:], 0.0)
            bdkT_pp.append(t)
            t = const.tile([128, 34 * 64], BF16, tag=f"bdvT{i}")
            nc.vector.memset(t[:], 0.0)
            for b in range(2):
                nc.vector.memset(
                    t[b * 64:(b + 1) * 64,
                      (32 + b) * 64:(32 + b + 1) * 64], 1.0)
            bdvT_pp.append(t)

        Copy = mybir.ActivationFunctionType.Copy
        Exp = mybir.ActivationFunctionType.Exp
        Mult = mybir.AluOpType.mult

        for mt in range(NMT):
            t0 = mt * MT
            # ---- load x megatile (already bf16) ----
            def load_x(dram, name):
                sbs = []
                for kc in range(8):
                    tb = p_xb.tile([128, MT], BF16, tag=f"x{name}",
                                   name=f"x{name}{mt}_{kc}")
                    nc.sync.dma_start(out=tb[:],
                                      in_=dram[kc * 128:(kc + 1) * 128,
                                               t0:t0 + MT])
                    sbs.append(tb)
                return sbs

            xq_sb = load_x(xq, "q")
            xk_sb = load_x(xk, "k")
            xv_sb = load_x(xv, "v")

            concatT = [p_cc.tile([128, MT], BF16, tag="cc",
                                 name=f"cc{mt}_{i}")
                       for i in range(8)]
            rcp64 = p_rcp.tile([64, MT], F32, tag="rcp64",
                               name=f"rcp64_{mt}")

            for tc4 in range(4):
                # ---- projections for this 128-token tile (token-major) ----
                qm = p_maj.tile([128, D], BF16, tag="qmaj",
                                name=f"qmaj{mt}_{tc4}")
                km = p_maj.tile([128, D], BF16, tag="kmaj",
                                name=f"kmaj{mt}_{tc4}")
                vm = p_maj.tile([128, D], BF16, tag="vmaj",
                                name=f"vmaj{mt}_{tc4}")
                for dst, xsb, wsb, is_k in ((qm, xq_sb, wq_sb, False),
                                            (km, xk_sb, wk_sb, True),
                                            (vm, xv_sb, wv_sb, False)):
                    pss = [ps_proj.tile([128, 512], F32, tag="psp",
                                        name=f"psp{mt}_{tc4}_{id(dst)}_{i}")
                           for i in range(2)]
                    for kc in range(8):
                        for oc2 in range(2):
                            nc.tensor.matmul(
                                out=pss[oc2][:],
                                lhsT=xsb[kc][:, tc4 * 128:(tc4 + 1) * 128],
                                rhs=wsb[kc][:, oc2 * 512:(oc2 + 1) * 512],
                                start=(kc == 0), stop=(kc == 7))
                    for oc2 in range(2):
                        dslice = dst[:, oc2 * 512:(oc2 + 1) * 512]
                        if is_k:
                            mcol = mt * 4 + tc4
                            nc.vector.tensor_scalar(
                                out=dslice, in0=pss[oc2][:],
                                scalar1=m32_sb[:, mcol:mcol + 1], scalar2=None,
                                op0=Mult)
                        else:
                            nc.scalar.activation(out=dslice, in_=pss[oc2][:],
                                                 func=Copy)

                if K_STAGE == 2 and (mt > 0 or tc4 > 0):
                    continue
                # ---- packing ----
                pp = tc4 % 2
                stqT, bdkT, bdvT = stqT_pp[pp], bdkT_pp[pp], bdvT_pp[pp]
                # stqT/bdkT via plain DRAM dumps (natural [t, f] order),
                # reloaded with arbitrary DRAM-side APs.
                sqD = p_stg.tile([128, D], BF16, tag="sqD",
                                 name=f"sqD{mt}_{tc4}")
                skD = p_stg.tile([128, D], BF16, tag="skD",
                                 name=f"skD{mt}_{tc4}")
                nc.sync.dma_start(out=sqD[:], in_=qm[:])
                nc.sync.dma_start(out=skD[:], in_=km[:])
                # stqT[(b,h),(g,i)] <- sqD[(b,g),(h,i)]: one call, out dim0
                # is the plain 32-partition dim.
                for b in range(2):
                    nc.sync.dma_start(
                        out=stqT[b * 16:(b + 1) * 16, :].rearrange(
                            "h (g i) -> h g i", i=64),
                        in_=sqD[b * 64:(b + 1) * 64, :].rearrange(
                            "g (h i) -> h g i", i=64))
                for b in range(2):
                    nc.sync.dma_start(
                        out=bdkT[b * 16:(b + 1) * 16, :].rearrange(
                            "h (g y j) -> h g y j", y=2, j=64)[:, :, b, :],
                        in_=skD[:].rearrange("(b g) (h j) -> b h g j",
                                             b=2, j=64)[b])
                # V: xbar-transpose vm 128-col windows (heads 2c,2c+1) into
                # VT[(hb,j), t], then partition-aligned copies into bdvT.
                vts = []
                for c in range(8):
                    vt = p_vt.tile([128, 128], BF16, tag="vt",
                                   name=f"vt{mt}_{tc4}_{c}")
                    nc.sync.dma_start(out=vt[:],
                                      in_=vm[:, c * 128:(c + 1) * 128],
                                      transpose=True)
                    vts.append(vt)
                bdvT_v = bdvT[:].rearrange("(b j) (c g) -> b j c g",
                                           b=2, g=64)
                for h in range(16):
                    vt = vts[h // 2]
                    hb = h % 2
                    for b in range(2):
                        nc.scalar.dma_start(
                            out=bdvT_v[b, :, b * 16 + h, :],
                            in_=vt[hb * 64:(hb + 1) * 64,
                                   b * 64:(b + 1) * 64])

                if K_STAGE == 2:
                    if mt == 0 and tc4 == 0:
                        nc.gpsimd.dma_start(out=out[0:32, :],
                                            in_=stqT[:, 0:2048])
                        nc.gpsimd.dma_start(out=out[32:64, :],
                                            in_=stqT[:, 2048:4096])
                        for r in range(4):
                            nc.gpsimd.dma_start(
                                out=out[64 + r * 32:96 + r * 32, :],
                                in_=bdkT[:, r * 2048:(r + 1) * 2048])
                        nc.gpsimd.dma_start(out=out[192:320, 0:2048],
                                            in_=bdvT[:, 0:2048])
                        nc.gpsimd.dma_start(out=out[320:448, 0:128],
                                            in_=bdvT[:, 2048:2176])
                        nc.gpsimd.dma_start(out=out[448:576, 0:1024],
                                            in_=qm[:])
                        nc.gpsimd.dma_start(out=out[576:704, 0:1024],
                                            in_=km[:])
                        nc.gpsimd.dma_start(out=out[704:832, 0:1024],
                                            in_=vm[:])
                    continue
                if K_STAGE == 2:
                    if mt == 0 and tc4 == 0:
                        nc.gpsimd.dma_start(out=out[0:32, :],
                                            in_=stqT[:, 0:2048])
                        nc.gpsimd.dma_start(out=out[32:64, :],
                                            in_=stqT[:, 2048:4096])
                        for r in range(4):
                            nc.gpsimd.dma_start(
                                out=out[64 + r * 32:96 + r * 32, :],
                                in_=bdkT[:, r * 2048:(r + 1) * 2048])
                        nc.gpsimd.dma_start(out=out[192:320, 0:2048],
                                            in_=bdvT[:, 0:2048])
                        nc.gpsimd.dma_start(out=out[320:448, 0:128],
                                            in_=bdvT[:, 2048:2176])
                        nc.gpsimd.dma_start(out=out[448:576, 0:1024],
                                            in_=qm[:])
                        nc.gpsimd.dma_start(out=out[576:704, 0:1024],
                                            in_=km[:])
                        nc.gpsimd.dma_start(out=out[704:832, 0:1024],
                                            in_=vm[:])
                    continue
                # ---- attention: E = bdkT^T stqT; A = exp(E); AV + Z ----
                hdw = p_hd.tile([64, 16, 128], BF16, tag="hdw",
                                name=f"hdw{mt}_{tc4}")  # [i][h][t]
                for batch in range(8):  # 16 tokens
                    bt = tc4 * 8 + batch
                    psE = ps_E.tile([128, 512], F32, tag="psE",
                                    name=f"psE{mt}_{bt}")
                    for g8 in range(8):
                        g = batch * 8 + g8      # token-pair index in tc4
                        nc.tensor.matmul(
                            out=psE[:, g8 * 64:(g8 + 1) * 64],
                            lhsT=bdkT[:, g * 128:(g + 1) * 128],
                            rhs=stqT[:, g * 64:(g + 1) * 64],
                            start=True, stop=True)
                    expE = p_exp.tile([128, 512], BF16, tag="expE",
                                      name=f"expE{mt}_{bt}")
                    nc.scalar.activation(out=expE[:], in_=psE[:], func=Exp)
                    ps2 = ps_2.tile([64, 272], F32, tag="ps2",
                                    name=f"ps2{mt}_{bt}")
                    bdvT_g = bdvT[:].rearrange("p (c g) -> p g c", g=64)
                    for g8 in range(8):
                        g = batch * 8 + g8
                        nc.tensor.matmul(
                            out=ps2[:, g8 * 34:(g8 + 1) * 34],
                            lhsT=expE[:, g8 * 64:(g8 + 1) * 64],
                            rhs=bdvT_g[:, g, :],
                            start=True, stop=True)
                    ps2v = ps2[:].rearrange("p (g c) -> p g c", c=34)
                    nc.vector.reciprocal(
                        rcp64[:, tc4 * 128:(tc4 + 1) * 128].rearrange(
                            "p (b g) -> p g b", b=2)[:, batch * 8:(batch + 1) * 8],
                        ps2v[:, :, 32:34])
                    # hdw[i, h, t=(b,g)] <- ps2 cols (g,(b,h))
                    nc.scalar.activation(
                        out=hdw[:].rearrange(
                            "p h (b g) -> p g b h", b=2)[:, batch * 8:(batch + 1) * 8],
                        in_=ps2v[:, :, 0:32].rearrange("p g (b h) -> p g b h",
                                                       h=16),
                        func=Copy)

                # concatT[kc][(hb,i), tc4-slice] <- hdw[i, 2kc+hb, t]
                for kc in range(8):
                    for hb in range(2):
                        nc.sync.dma_start(
                            out=concatT[kc][hb * 64:(hb + 1) * 64,
                                            tc4 * 128:(tc4 + 1) * 128],
                            in_=hdw[:, 2 * kc + hb, :])

            if K_STAGE == 2:
                continue
            if K_STAGE == 2:
                continue
            # ---- normalize + output projection ----
            rcp128 = p_rcp.tile([128, MT], F32, tag="rcp128",
                                name=f"rcp128_{mt}")
            nc.vector.tensor_copy(rcp128[0:64, :], rcp64[:])
            nc.sync.dma_start(out=rcp128[64:128, :], in_=rcp64[:])
            ccb = []
            for kc in range(8):
                cb = p_cc.tile([128, MT], BF16, tag="ccb",
                               name=f"ccb{mt}_{kc}")
                nc.vector.tensor_tensor(out=cb[:], in0=concatT[kc][:],
                                        in1=rcp128[:], op=Mult)
                ccb.append(cb)
            for oc in range(8):
                ps = ps_proj.tile([128, 512], F32, tag="psp",
                                  name=f"pspo{mt}_{oc}")
                for kc in range(8):
                    nc.tensor.matmul(out=ps[:],
                                     lhsT=wo_sb[kc][:, oc * 128:(oc + 1) * 128],
                                     rhs=ccb[kc][:],
                                     start=(kc == 0), stop=(kc == 7))
                ot = p_outT.tile([128, MT], F32, tag="outT",
                                 name=f"outT{mt}_{oc}")
                nc.scalar.activation(out=ot[:], in_=ps[:], func=Copy)
                nc.sync.dma_start(out=out[oc * 128:(oc + 1) * 128, t0:t0 + MT],
                                  in_=ot[:])
    nc.compile()
    return nc


_NC_CACHE = None


def _get_nc():
    global _NC_CACHE
    if _NC_CACHE is None:
        _NC_CACHE = build_nc()
    return _NC_CACHE


def _host_prep(queries, keys, values, mask, Wq, Wk, Wv, Wo):
    """Build the 8 per-core input maps."""
    import ml_dtypes
    bf = lambda a: np.ascontiguousarray(a).astype(ml_dtypes.bfloat16)
    fq = bf(queries.reshape(N * T, D).T)  # [D, 16384] bf16
    fk = bf(keys.reshape(N * T, D).T)
    fv = bf(values.reshape(N * T, D).T)
    fm = mask.reshape(N * T).astype(np.float32) / 32.0

    wq_h = bf(Wq.T)
    wk_h = bf(Wk.T)
    wv_h = bf(Wv.T)  # natural (h,j) column order: xbar windows = head pairs
    kpp = np.arange(D)
    perm = (kpp % 64) * 16 + (kpp // 64)  # k''=h*64+i -> source row i*16+h
    wo_h = bf(Wo.T[perm])

    in_maps = []
    for c in range(N_CORES):
        s = slice(c * TOK, (c + 1) * TOK)
        in_maps.append({
            "xq": np.ascontiguousarray(fq[:, s]),
            "xk": np.ascontiguousarray(fk[:, s]),
            "xv": np.ascontiguousarray(fv[:, s]),
            "wq": wq_h, "wk": wk_h, "wv": wv_h, "wo": wo_h,
            "m32": np.ascontiguousarray(fm[s].reshape(TOK // 128, 128).T),
        })
    return in_maps


def kernel(queries, keys, values, mask, Wq, Wk, Wv, Wo, _trace=False, _tmpdir=None):
    queries = np.asarray(queries, dtype=np.float32)
    keys = np.asarray(keys, dtype=np.float32)
    values = np.asarray(values, dtype=np.float32)
    mask = np.asarray(mask)
    in_maps = _host_prep(queries, keys, values, mask,
                         np.asarray(Wq, np.float32), np.asarray(Wk, np.float32),
                         np.asarray(Wv, np.float32), np.asarray(Wo, np.float32))
    nc = _get_nc()
    res = run_bass_kernel_spmd(nc, in_maps, core_ids=list(range(N_CORES)),
                               trace=_trace, tmpdir=_tmpdir)
    outs = []
    for c in range(N_CORES):
        outs.append(np.asarray(res.results[c]["out"]).T)  # [TOK, D]
    full = np.concatenate(outs, axis=0).reshape(N, T, D)
    kernel.last_exec_time_ns = res.exec_time_ns
    return full


# revision 20
# speedup vs baseline: 1.3697x; 1.3697x over previous
"""Trainium2 Bass kernel for nn_MultiHeadAttention_42640435315371.

Data-parallel over 8 NeuronCores: each core handles 2048 of the 16384
(n*t) tokens; the four d_model x d_model weights are replicated (shipped
bf16, pre-transposed/permuted on host).

Math notes (matching reference.py exactly):
  - energy_t = Qh_t^T Kh_t / 32 per token (token-local "attention");
    the 1/32 scale and the mask are folded into K as K * mask/32, so a
    masked token yields an all-zero energy matrix -> softmax = uniform
    1/64, identical to softmax of a constant -1e20 row.
  - energies are tiny (|E| < ~1), so exp() needs no max-subtraction.
  - concat order is (d_head, head); Wo's columns are permuted on host so
    the device can emit rows k = h*64 + i.

v2: inputs shipped bf16 (halves input HBM); all shuffle staging through
DRAM replaced by direct SBUF->SBUF DMAs with composed access patterns;
head outputs staged in a per-tc4 [64, 16, 128] tile so the concat
reassembly DMA uses 512B runs.
"""

import os
import os
import numpy as np

import concourse.bass as bass
import concourse.mybir as mybir
from concourse import bacc
from concourse.tile import TileContext
from concourse.bass_utils import run_bass_kernel_spmd

F32 = mybir.dt.float32
BF16 = mybir.dt.bfloat16

K_STAGE = int(os.environ.get("K_STAGE", "99"))
K_STAGE = int(os.environ.get("K_STAGE", "99"))
N_CORES = 8
N, T, D, H, DH = 4, 4096, 1024, 16, 64
TOK = (N * T) // N_CORES  # 2048 tokens per core
MT = 512                  # megatile tokens
NMT = TOK // MT


def build_nc():
    nc = bacc.Bacc("TRN2", target_bir_lowering=False, debug=False,
                   num_devices=N_CORES)
    xq = nc.declare_dram_parameter("xq", [D, TOK], BF16, isOutput=False)
    xk = nc.declare_dram_parameter("xk", [D, TOK], BF16, isOutput=False)
    xv = nc.declare_dram_parameter("xv", [D, TOK], BF16, isOutput=False)
    wq = nc.declare_dram_parameter("wq", [D, D], BF16, isOutput=False)
    wk = nc.declare_dram_parameter("wk", [D, D], BF16, isOutput=False)
    wv = nc.declare_dram_parameter("wv", [D, D], BF16, isOutput=False)
    wo = nc.declare_dram_parameter("wo", [D, D], BF16, isOutput=False)
    m32 = nc.declare_dram_parameter("m32", [128, TOK // 128], F32, isOutput=False)
    out = nc.declare_dram_parameter("out", [D, TOK], F32, isOutput=True)

    from contextlib import ExitStack
    with TileContext(nc) as tc, ExitStack() as ctx:
        const = ctx.enter_context(tc.tile_pool(name="const", bufs=1))
        p_xb = ctx.enter_context(tc.tile_pool(name="xb", bufs=8))
        p_maj = ctx.enter_context(tc.tile_pool(name="maj", bufs=4))
        p_vd = ctx.enter_context(tc.tile_pool(name="vd", bufs=8))
        p_cc = ctx.enter_context(tc.tile_pool(name="cc", bufs=8))
        p_exp = ctx.enter_context(tc.tile_pool(name="expp", bufs=3))
        p_hd = ctx.enter_context(tc.tile_pool(name="hd", bufs=2))
        p_rcp = ctx.enter_context(tc.tile_pool(name="rcp", bufs=1))
        p_outT = ctx.enter_context(tc.tile_pool(name="outT", bufs=2))
        ps_proj = ctx.enter_context(tc.tile_pool(name="psp", bufs=2, space="PSUM"))
        ps_E = ctx.enter_context(tc.tile_pool(name="psE", bufs=3, space="PSUM"))
        ps_2 = ctx.enter_context(tc.tile_pool(name="ps2", bufs=2, space="PSUM"))
        p_stg = ctx.enter_context(tc.tile_pool(name="stg", bufs=2, space="DRAM"))

        # ---- static tiles ----
        def load_w(name, dram):
            tiles = []
            for i in range(8):
                t = const.tile([128, D], BF16, tag=f"{name}{i}")
                nc.sync.dma_start(out=t[:], in_=dram[i * 128:(i + 1) * 128, :])
                tiles.append(t)
            return tiles

        wq_sb, wk_sb, wv_sb, wo_sb = (load_w(n, d) for n, d in
                                      (("wq", wq), ("wk", wk), ("wv", wv), ("wo", wo)))
        m32_sb = const.tile([128, TOK // 128], F32, tag="m32")
        nc.sync.dma_start(out=m32_sb[:], in_=m32[:])

        # Packed per-tc4 tiles (2x ping-pong; zeros/ones written once).
        # stqT [32=(b2,h16), 64g*64i]: stqT[(b,h),(g,i)] = Q[2g+b, h*64+i]
        # bdkT [32=(b2,h16), 64g*2y*64j]: nonzero strip y==b holds
        #   K[2g+b, h*64+j] (block-diagonal in (b,y)).
        # bdvT [128=(b2,j64), 64g*34c]: cols c=b'*16+h hold V[2g+b, j, h]
        #   for b'==b; cols 32+b' hold ones on partition-half b'.
        stqT_pp, bdkT_pp, bdvT_pp = [], [], []
        for i in range(2):
            t = const.tile([32, 64 * 64], BF16, tag=f"stqT{i}")
            stqT_pp.append(t)
            t = const.tile([32, 64 * 128], BF16, tag=f"bdkT{i}")
            nc.vector.memset(t[:], 0.0)
            bdkT_pp.append(t)
            t = const.tile([128, 34 * 64], BF16, tag=f"bdvT{i}")
            nc.vector.memset(t[:], 0.0)
            for b in range(2):
                nc.vector.memset(
                    t[b * 64:(b + 1) * 64,
                      (32 + b) * 64:(32 + b + 1) * 64], 1.0)
            bdvT_pp.append(t)

        Copy = mybir.ActivationFunctionType.Copy
        Exp = mybir.ActivationFunctionType.Exp
        Mult = mybir.AluOpType.mult

        for mt in range(NMT):
            t0 = mt * MT
            # ---- load x megatile (already bf16) ----
            def load_x(dram, name):
                sbs = []
                for kc in range(8):
                    tb = p_xb.tile([128, MT], BF16, tag=f"x{name}",
                                   name=f"x{name}{mt}_{kc}")
                    nc.sync.dma_start(out=tb[:],
                                      in_=dram[kc * 128:(kc + 1) * 128,
                                               t0:t0 + MT])
                    sbs.append(tb)
                return sbs

            xq_sb = load_x(xq, "q")
            xk_sb = load_x(xk, "k")
            xv_sb = load_x(xv, "v")

            concatT = [p_cc.tile([128, MT], BF16, tag="cc",
                                 name=f"cc{mt}_{i}")
                       for i in range(8)]
            rcp64 = p_rcp.tile([64, MT], F32, tag="rcp64",
                               name=f"rcp64_{mt}")

            qm_list, km_list = [], []
            for tc4 in range(4):
                # ---- projections for this 128-token tile (token-major) ----
                qm = p_maj.tile([128, D], BF16, tag="qmaj",
                                name=f"qmaj{mt}_{tc4}")
                km = p_maj.tile([128, D], BF16, tag="kmaj",
                                name=f"kmaj{mt}_{tc4}")
                for dst, xsb, wsb, is_k in ((qm, xq_sb, wq_sb, False),
                                            (km, xk_sb, wk_sb, True)):
                    pss = [ps_proj.tile([128, 512], F32, tag="psp",
                                        name=f"psp{mt}_{tc4}_{id(dst)}_{i}")
                           for i in range(2)]
                    for kc in range(8):
                        for oc2 in range(2):
                            nc.tensor.matmul(
                                out=pss[oc2][:],
                                lhsT=xsb[kc][:, tc4 * 128:(tc4 + 1) * 128],
                                rhs=wsb[kc][:, oc2 * 512:(oc2 + 1) * 512],
                                start=(kc == 0), stop=(kc == 7))
                    for oc2 in range(2):
                        dslice = dst[:, oc2 * 512:(oc2 + 1) * 512]
                        if is_k:
                            mcol = mt * 4 + tc4
                            nc.vector.tensor_scalar(
                                out=dslice, in0=pss[oc2][:],
                                scalar1=m32_sb[:, mcol:mcol + 1], scalar2=None,
                                op0=Mult)
                        else:
                            nc.scalar.activation(out=dslice, in_=pss[oc2][:],
                                                 func=Copy)
                qm_list.append(qm)
                km_list.append(km)
            # ---- V projection, D-major: vD[c][(hb,j), t] over the full MT.
            # lhsT = wv chunk [din,128 dout], rhs = xv [din, 512 tok].
            vD = []
            for c in range(8):
                psv = ps_proj.tile([128, 512], F32, tag="psp",
                                   name=f"psv{mt}_{c}")
                for kc in range(8):
                    nc.tensor.matmul(
                        out=psv[:],
                        lhsT=wv_sb[kc][:, c * 128:(c + 1) * 128],
                        rhs=xv_sb[kc][:],
                        start=(kc == 0), stop=(kc == 7))
                vc = p_vd.tile([128, MT], BF16, tag="vD",
                                name=f"vD{mt}_{c}")
                nc.scalar.activation(out=vc[:], in_=psv[:], func=Copy)
                vD.append(vc)

            for tc4 in range(4):
                if K_STAGE == 2 and (mt > 0 or tc4 > 0):
                    continue
                qm = qm_list[tc4]
                km = km_list[tc4]
                # ---- packing ----
                pp = tc4 % 2
                stqT, bdkT, bdvT = stqT_pp[pp], bdkT_pp[pp], bdvT_pp[pp]
                # stqT/bdkT via plain DRAM dumps (natural [t, f] order),
                # reloaded with arbitrary DRAM-side APs.
                sqD = p_stg.tile([128, D], BF16, tag="sqD",
                                 name=f"sqD{mt}_{tc4}")
                skD = p_stg.tile([128, D], BF16, tag="skD",
                                 name=f"skD{mt}_{tc4}")
                nc.sync.dma_start(out=sqD[:], in_=qm[:])
                nc.sync.dma_start(out=skD[:], in_=km[:])
                # stqT[(b,h),(g,i)] <- sqD[(b,g),(h,i)]: one call, out dim0
                # is the plain 32-partition dim.
                for b in range(2):
                    nc.sync.dma_start(
                        out=stqT[b * 16:(b + 1) * 16, :].rearrange(
                            "h (g i) -> h g i", i=64),
                        in_=sqD[b * 64:(b + 1) * 64, :].rearrange(
                            "g (h i) -> h g i", i=64))
                for b in range(2):
                    nc.sync.dma_start(
                        out=bdkT[b * 16:(b + 1) * 16, :].rearrange(
                            "h (g y j) -> h g y j", y=2, j=64)[:, :, b, :],
                        in_=skD[:].rearrange("(b g) (h j) -> b h g j",
                                             b=2, j=64)[b])
                # bdvT blocks from D-major vD chunks: plain ranges.
                for h in range(16):
                    vc = vD[h // 2]
                    hb = h % 2
                    for b in range(2):
                        t0c = tc4 * 128 + b * 64
                        nc.scalar.dma_start(
                            out=bdvT[b * 64:(b + 1) * 64,
                                     (b * 16 + h) * 64:(b * 16 + h + 1) * 64],
                            in_=vc[hb * 64:(hb + 1) * 64, t0c:t0c + 64])

                if K_STAGE == 2:
                    if mt == 0 and tc4 == 0:
                        nc.gpsimd.dma_start(out=out[0:32, :],
                                            in_=stqT[:, 0:2048])
                        nc.gpsimd.dma_start(out=out[32:64, :],
                                            in_=stqT[:, 2048:4096])
                        for r in range(4):
                            nc.gpsimd.dma_start(
                                out=out[64 + r * 32:96 + r * 32, :],
                                in_=bdkT[:, r * 2048:(r + 1) * 2048])
                        nc.gpsimd.dma_start(out=out[192:320, 0:2048],
                                            in_=bdvT[:, 0:2048])
                        nc.gpsimd.dma_start(out=out[320:448, 0:128],
                                            in_=bdvT[:, 2048:2176])
                        nc.gpsimd.dma_start(out=out[448:576, 0:1024],
                                            in_=qm[:])
                        nc.gpsimd.dma_start(out=out[576:704, 0:1024],
                                            in_=km[:])
                        nc.gpsimd.dma_start(out=out[704:832, 0:512],
                                            in_=vD[0][:])
                    continue
                if K_STAGE == 2:
                    if mt == 0 and tc4 == 0:
                        nc.gpsimd.dma_start(out=out[0:32, :],
                                            in_=stqT[:, 0:2048])
                        nc.gpsimd.dma_start(out=out[32:64, :],
                                            in_=stqT[:, 2048:4096])
                        for r in range(4):
                            nc.gpsimd.dma_start(
                                out=out[64 + r * 32:96 + r * 32, :],
                                in_=bdkT[:, r * 2048:(r + 1) * 2048])
                        nc.gpsimd.dma_start(out=out[192:320, 0:2048],
                                            in_=bdvT[:, 0:2048])
                        nc.gpsimd.dma_start(out=out[320:448, 0:128],
                                            in_=bdvT[:, 2048:2176])
                        nc.gpsimd.dma_start(out=out[448:576, 0:1024],
                                            in_=qm[:])
                        nc.gpsimd.dma_start(out=out[576:704, 0:1024],
                                            in_=km[:])
                        nc.gpsimd.dma_start(out=out[704:832, 0:512],
                                            in_=vD[0][:])
                    continue
                # ---- attention: E = bdkT^T stqT; A = exp(E); AV + Z ----
                hdw = p_hd.tile([64, 16, 128], BF16, tag="hdw",
                                name=f"hdw{mt}_{tc4}")  # [i][h][t]
                for batch in range(8):  # 16 tokens
                    bt = tc4 * 8 + batch
                    psE = ps_E.tile([128, 512], F32, tag="psE",
                                    name=f"psE{mt}_{bt}")
                    for g8 in range(8):
                        g = batch * 8 + g8      # token-pair index in tc4
                        nc.tensor.matmul(
                            out=psE[:, g8 * 64:(g8 + 1) * 64],
                            lhsT=bdkT[:, g * 128:(g + 1) * 128],
                            rhs=stqT[:, g * 64:(g + 1) * 64],
                            start=True, stop=True)
                    expE = p_exp.tile([128, 512], BF16, tag="expE",
                                      name=f"expE{mt}_{bt}")
                    nc.scalar.activation(out=expE[:], in_=psE[:], func=Exp)
                    ps2 = ps_2.tile([64, 272], F32, tag="ps2",
                                    name=f"ps2{mt}_{bt}")
                    bdvT_g = bdvT[:].rearrange("p (c g) -> p g c", g=64)
                    for g8 in range(8):
                        g = batch * 8 + g8
                        nc.tensor.matmul(
                            out=ps2[:, g8 * 34:(g8 + 1) * 34],
                            lhsT=expE[:, g8 * 64:(g8 + 1) * 64],
                            rhs=bdvT_g[:, g, :],
                            start=True, stop=True)
                    ps2v = ps2[:].rearrange("p (g c) -> p g c", c=34)
                    nc.vector.reciprocal(
                        rcp64[:, tc4 * 128:(tc4 + 1) * 128].rearrange(
                            "p (b g) -> p g b", b=2)[:, batch * 8:(batch + 1) * 8],
                        ps2v[:, :, 32:34])
                    # hdw[i, h, t=(b,g)] <- ps2 cols (g,(b,h))
                    nc.vector.tensor_copy(
                        hdw[:].rearrange(
                            "p h (b g) -> p g b h", b=2)[:, batch * 8:(batch + 1) * 8],
                        ps2v[:, :, 0:32].rearrange("p g (b h) -> p g b h",
                                                   h=16))

                # concatT[kc][(hb,i), tc4-slice] <- hdw[i, 2kc+hb, t]
                for kc in range(8):
                    for hb in range(2):
                        nc.sync.dma_start(
                            out=concatT[kc][hb * 64:(hb + 1) * 64,
                                            tc4 * 128:(tc4 + 1) * 128],
                            in_=hdw[:, 2 * kc + hb, :])

            if K_STAGE == 2:
                continue
            if K_STAGE == 2:
                continue
            # ---- normalize + output projection ----
            rcp128 = p_rcp.tile([128, MT], F32, tag="rcp128",
                                name=f"rcp128_{mt}")
            nc.vector.tensor_copy(rcp128[0:64, :], rcp64[:])
            nc.sync.dma_start(out=rcp128[64:128, :], in_=rcp64[:])
            ccb = []
            for kc in range(8):
                cb = p_cc.tile([128, MT], BF16, tag="ccb",
                               name=f"ccb{mt}_{kc}")
                nc.vector.tensor_tensor(out=cb[:], in0=concatT[kc][:],
                                        in1=rcp128[:], op=Mult)
                ccb.append(cb)
            for oc in range(8):
                ps = ps_proj.tile([128, 512], F32, tag="psp",
                                  name=f"pspo{mt}_{oc}")
                for kc in range(8):
                    nc.tensor.matmul(out=ps[:],
                                     lhsT=wo_sb[kc][:, oc * 128:(oc + 1) * 128],
                                     rhs=ccb[kc][:],
                                     start=(kc == 0), stop=(kc == 7))
                ot = p_outT.tile([128, MT], F32, tag="outT",
                                 name=f"outT{mt}_{oc}")
                nc.scalar.activation(out=ot[:], in_=ps[:], func=Copy)
                nc.sync.dma_start(out=out[oc * 128:(oc + 1) * 128, t0:t0 + MT],
                                  in_=ot[:])
    nc.compile()
    return nc


_NC_CACHE = None


def _get_nc():
    global _NC_CACHE
    if _NC_CACHE is None:
        _NC_CACHE = build_nc()
    return _NC_CACHE


def _host_prep(queries, keys, values, mask, Wq, Wk, Wv, Wo):
    """Build the 8 per-core input maps."""
    import ml_dtypes
    bf = lambda a: np.ascontiguousarray(a).astype(ml_dtypes.bfloat16)
    fq = bf(queries.reshape(N * T, D).T)  # [D, 16384] bf16
    fk = bf(keys.reshape(N * T, D).T)
    fv = bf(values.reshape(N * T, D).T)
    fm = mask.reshape(N * T).astype(np.float32) / 32.0

    wq_h = bf(Wq.T)
    wk_h = bf(Wk.T)
    wv_h = bf(Wv.T)  # natural (h,j) column order: xbar windows = head pairs
    kpp = np.arange(D)
    perm = (kpp % 64) * 16 + (kpp // 64)  # k''=h*64+i -> source row i*16+h
    wo_h = bf(Wo.T[perm])

    in_maps = []
    for c in range(N_CORES):
        s = slice(c * TOK, (c + 1) * TOK)
        in_maps.append({
            "xq": np.ascontiguousarray(fq[:, s]),
            "xk": np.ascontiguousarray(fk[:, s]),
            "xv": np.ascontiguousarray(fv[:, s]),
            "wq": wq_h, "wk": wk_h, "wv": wv_h, "wo": wo_h,
            "m32": np.ascontiguousarray(fm[s].reshape(TOK // 128, 128).T),
        })
    return in_maps


def kernel(queries, keys, values, mask, Wq, Wk, Wv, Wo, _trace=False, _tmpdir=None):
    queries = np.asarray(queries, dtype=np.float32)
    keys = np.asarray(keys, dtype=np.float32)
    values = np.asarray(values, dtype=np.float32)
    mask = np.asarray(mask)
    in_maps = _host_prep(queries, keys, values, mask,
                         np.asarray(Wq, np.float32), np.asarray(Wk, np.float32),
                         np.asarray(Wv, np.float32), np.asarray(Wo, np.float32))
    nc = _get_nc()
    res = run_bass_kernel_spmd(nc, in_maps, core_ids=list(range(N_CORES)),
                               trace=_trace, tmpdir=_tmpdir)
    outs = []
    for c in range(N_CORES):
        outs.append(np.asarray(res.results[c]["out"]).T)  # [TOK, D]
    full = np.concatenate(outs, axis=0).reshape(N, T, D)
    kernel.last_exec_time_ns = res.exec_time_ns
    return full


# revision 21
# speedup vs baseline: 1.4630x; 1.0682x over previous
"""Trainium2 Bass kernel for nn_MultiHeadAttention_42640435315371.

Data-parallel over 8 NeuronCores: each core handles 2048 of the 16384
(n*t) tokens; the four d_model x d_model weights are replicated (shipped
bf16, pre-transposed/permuted on host).

Math notes (matching reference.py exactly):
  - energy_t = Qh_t^T Kh_t / 32 per token (token-local "attention");
    the 1/32 scale and the mask are folded into K as K * mask/32, so a
    masked token yields an all-zero energy matrix -> softmax = uniform
    1/64, identical to softmax of a constant -1e20 row.
  - energies are tiny (|E| < ~1), so exp() needs no max-subtraction.
  - concat order is (d_head, head); Wo's columns are permuted on host so
    the device can emit rows k = h*64 + i.

v2: inputs shipped bf16 (halves input HBM); all shuffle staging through
DRAM replaced by direct SBUF->SBUF DMAs with composed access patterns;
head outputs staged in a per-tc4 [64, 16, 128] tile so the concat
reassembly DMA uses 512B runs.
"""

import os
import os
import numpy as np

import concourse.bass as bass
import concourse.mybir as mybir
from concourse import bacc
from concourse.tile import TileContext
from concourse.bass_utils import run_bass_kernel_spmd

F32 = mybir.dt.float32
BF16 = mybir.dt.bfloat16

K_STAGE = int(os.environ.get("K_STAGE", "99"))
K_STAGE = int(os.environ.get("K_STAGE", "99"))
N_CORES = 8
N, T, D, H, DH = 4, 4096, 1024, 16, 64
TOK = (N * T) // N_CORES  # 2048 tokens per core
MT = 512                  # megatile tokens
NMT = TOK // MT


def build_nc():
    nc = bacc.Bacc("TRN2", target_bir_lowering=False, debug=False,
                   num_devices=N_CORES)
    xq = nc.declare_dram_parameter("xq", [D, TOK], BF16, isOutput=False)
    xk = nc.declare_dram_parameter("xk", [D, TOK], BF16, isOutput=False)
    xv = nc.declare_dram_parameter("xv", [D, TOK], BF16, isOutput=False)
    wq = nc.declare_dram_parameter("wq", [D, D], BF16, isOutput=False)
    wk = nc.declare_dram_parameter("wk", [D, D], BF16, isOutput=False)
    wv = nc.declare_dram_parameter("wv", [D, D], BF16, isOutput=False)
    wo = nc.declare_dram_parameter("wo", [D, D], BF16, isOutput=False)
    m32 = nc.declare_dram_parameter("m32", [128, TOK // 128], F32, isOutput=False)
    out = nc.declare_dram_parameter("out", [D, TOK], F32, isOutput=True)

    from contextlib import ExitStack
    with TileContext(nc) as tc, ExitStack() as ctx:
        const = ctx.enter_context(tc.tile_pool(name="const", bufs=1))
        p_xb = ctx.enter_context(tc.tile_pool(name="xb", bufs=8))
        p_maj = ctx.enter_context(tc.tile_pool(name="maj", bufs=4))
        p_vd = ctx.enter_context(tc.tile_pool(name="vd", bufs=8))
        p_cc = ctx.enter_context(tc.tile_pool(name="cc", bufs=8))
        p_exp = ctx.enter_context(tc.tile_pool(name="expp", bufs=3))
        p_hd = ctx.enter_context(tc.tile_pool(name="hd", bufs=2))
        p_rcp = ctx.enter_context(tc.tile_pool(name="rcp", bufs=1))
        p_outT = ctx.enter_context(tc.tile_pool(name="outT", bufs=2))
        ps_proj = ctx.enter_context(tc.tile_pool(name="psp", bufs=2, space="PSUM"))
        ps_E = ctx.enter_context(tc.tile_pool(name="psE", bufs=2, space="PSUM"))
        ps_o = ctx.enter_context(tc.tile_pool(name="pso", bufs=2, space="PSUM"))
        ps_2 = ctx.enter_context(tc.tile_pool(name="ps2", bufs=2, space="PSUM"))
        p_stg = ctx.enter_context(tc.tile_pool(name="stg", bufs=2, space="DRAM"))

        # ---- static tiles ----
        def load_w(name, dram):
            tiles = []
            for i in range(8):
                t = const.tile([128, D], BF16, tag=f"{name}{i}")
                nc.sync.dma_start(out=t[:], in_=dram[i * 128:(i + 1) * 128, :])
                tiles.append(t)
            return tiles

        wq_sb, wk_sb, wv_sb, wo_sb = (load_w(n, d) for n, d in
                                      (("wq", wq), ("wk", wk), ("wv", wv), ("wo", wo)))
        m32_sb = const.tile([128, TOK // 128], F32, tag="m32")
        nc.sync.dma_start(out=m32_sb[:], in_=m32[:])

        # Packed per-tc4 tiles (2x ping-pong; zeros/ones written once).
        # stqT [32=(b2,h16), 64g*64i]: stqT[(b,h),(g,i)] = Q[2g+b, h*64+i]
        # bdkT [32=(b2,h16), 64g*2y*64j]: nonzero strip y==b holds
        #   K[2g+b, h*64+j] (block-diagonal in (b,y)).
        # bdvT [128=(b2,j64), 64g*34c]: cols c=b'*16+h hold V[2g+b, j, h]
        #   for b'==b; cols 32+b' hold ones on partition-half b'.
        stqT_pp, bdkT_pp, bdvT_pp = [], [], []
        for i in range(2):
            t = const.tile([32, 64 * 64], BF16, tag=f"stqT{i}")
            stqT_pp.append(t)
            t = const.tile([32, 64 * 128], BF16, tag=f"bdkT{i}")
            nc.vector.memset(t[:], 0.0)
            bdkT_pp.append(t)
            t = const.tile([128, 34 * 64], BF16, tag=f"bdvT{i}")
            nc.vector.memset(t[:], 0.0)
            for b in range(2):
                nc.vector.memset(
                    t[b * 64:(b + 1) * 64,
                      (32 + b) * 64:(32 + b + 1) * 64], 1.0)
            bdvT_pp.append(t)

        Copy = mybir.ActivationFunctionType.Copy
        Exp = mybir.ActivationFunctionType.Exp
        Mult = mybir.AluOpType.mult

        for mt in range(NMT):
            t0 = mt * MT
            # ---- load x megatile (already bf16) ----
            def load_x(dram, name):
                sbs = []
                for kc in range(8):
                    tb = p_xb.tile([128, MT], BF16, tag=f"x{name}",
                                   name=f"x{name}{mt}_{kc}")
                    nc.sync.dma_start(out=tb[:],
                                      in_=dram[kc * 128:(kc + 1) * 128,
                                               t0:t0 + MT])
                    sbs.append(tb)
                return sbs

            xq_sb = load_x(xq, "q")
            xk_sb = load_x(xk, "k")
            xv_sb = load_x(xv, "v")

            concatT = [p_cc.tile([128, MT], BF16, tag="cc",
                                 name=f"cc{mt}_{i}")
                       for i in range(8)]
            rcp64 = p_rcp.tile([64, MT], F32, tag="rcp64",
                               name=f"rcp64_{mt}")

            qm_list, km_list = [], []
            for tc4 in range(4):
                # ---- projections for this 128-token tile (token-major) ----
                qm = p_maj.tile([128, D], BF16, tag="qmaj",
                                name=f"qmaj{mt}_{tc4}")
                km = p_maj.tile([128, D], BF16, tag="kmaj",
                                name=f"kmaj{mt}_{tc4}")
                for dst, xsb, wsb, is_k in ((qm, xq_sb, wq_sb, False),
                                            (km, xk_sb, wk_sb, True)):
                    pss = [ps_proj.tile([128, 512], F32, tag="psp",
                                        name=f"psp{mt}_{tc4}_{id(dst)}_{i}")
                           for i in range(2)]
                    for kc in range(8):
                        for oc2 in range(2):
                            nc.tensor.matmul(
                                out=pss[oc2][:],
                                lhsT=xsb[kc][:, tc4 * 128:(tc4 + 1) * 128],
                                rhs=wsb[kc][:, oc2 * 512:(oc2 + 1) * 512],
                                start=(kc == 0), stop=(kc == 7))
                    for oc2 in range(2):
                        dslice = dst[:, oc2 * 512:(oc2 + 1) * 512]
                        if is_k:
                            mcol = mt * 4 + tc4
                            nc.vector.tensor_scalar(
                                out=dslice, in0=pss[oc2][:],
                                scalar1=m32_sb[:, mcol:mcol + 1], scalar2=None,
                                op0=Mult)
                        else:
                            nc.scalar.activation(out=dslice, in_=pss[oc2][:],
                                                 func=Copy)
                qm_list.append(qm)
                km_list.append(km)
            # ---- V projection, D-major: vD[c][(hb,j), t] over the full MT.
            # lhsT = wv chunk [din,128 dout], rhs = xv [din, 512 tok].
            vD = []
            for c in range(8):
                psv = ps_proj.tile([128, 512], F32, tag="psp",
                                   name=f"psv{mt}_{c}")
                for kc in range(8):
                    nc.tensor.matmul(
                        out=psv[:],
                        lhsT=wv_sb[kc][:, c * 128:(c + 1) * 128],
                        rhs=xv_sb[kc][:],
                        start=(kc == 0), stop=(kc == 7))
                vc = p_vd.tile([128, MT], BF16, tag="vD",
                                name=f"vD{mt}_{c}")
                nc.scalar.activation(out=vc[:], in_=psv[:], func=Copy)
                vD.append(vc)

            for tc4 in range(4):
                if K_STAGE == 2 and (mt > 0 or tc4 > 0):
                    continue
                qm = qm_list[tc4]
                km = km_list[tc4]
                # ---- packing ----
                pp = tc4 % 2
                stqT, bdkT, bdvT = stqT_pp[pp], bdkT_pp[pp], bdvT_pp[pp]
                # stqT/bdkT via plain DRAM dumps (natural [t, f] order),
                # reloaded with arbitrary DRAM-side APs.
                sqD = p_stg.tile([128, D], BF16, tag="sqD",
                                 name=f"sqD{mt}_{tc4}")
                skD = p_stg.tile([128, D], BF16, tag="skD",
                                 name=f"skD{mt}_{tc4}")
                nc.sync.dma_start(out=sqD[:], in_=qm[:])
                nc.sync.dma_start(out=skD[:], in_=km[:])
                # stqT[(b,h),(g,i)] <- sqD[(b,g),(h,i)]: one call, out dim0
                # is the plain 32-partition dim.
                for b in range(2):
                    nc.sync.dma_start(
                        out=stqT[b * 16:(b + 1) * 16, :].rearrange(
                            "h (g i) -> h g i", i=64),
                        in_=sqD[b * 64:(b + 1) * 64, :].rearrange(
                            "g (h i) -> h g i", i=64))
                for b in range(2):
                    nc.sync.dma_start(
                        out=bdkT[b * 16:(b + 1) * 16, :].rearrange(
                            "h (g y j) -> h g y j", y=2, j=64)[:, :, b, :],
                        in_=skD[:].rearrange("(b g) (h j) -> b h g j",
                                             b=2, j=64)[b])
                # bdvT blocks from D-major vD chunks: plain ranges.
                for h in range(16):
                    vc = vD[h // 2]
                    hb = h % 2
                    for b in range(2):
                        t0c = tc4 * 128 + b * 64
                        nc.scalar.dma_start(
                            out=bdvT[b * 64:(b + 1) * 64,
                                     (b * 16 + h) * 64:(b * 16 + h + 1) * 64],
                            in_=vc[hb * 64:(hb + 1) * 64, t0c:t0c + 64])

                if K_STAGE == 2:
                    if mt == 0 and tc4 == 0:
                        nc.gpsimd.dma_start(out=out[0:32, :],
                                            in_=stqT[:, 0:2048])
                        nc.gpsimd.dma_start(out=out[32:64, :],
                                            in_=stqT[:, 2048:4096])
                        for r in range(4):
                            nc.gpsimd.dma_start(
                                out=out[64 + r * 32:96 + r * 32, :],
                                in_=bdkT[:, r * 2048:(r + 1) * 2048])
                        nc.gpsimd.dma_start(out=out[192:320, 0:2048],
                                            in_=bdvT[:, 0:2048])
                        nc.gpsimd.dma_start(out=out[320:448, 0:128],
                                            in_=bdvT[:, 2048:2176])
                        nc.gpsimd.dma_start(out=out[448:576, 0:1024],
                                            in_=qm[:])
                        nc.gpsimd.dma_start(out=out[576:704, 0:1024],
                                            in_=km[:])
                        nc.gpsimd.dma_start(out=out[704:832, 0:512],
                                            in_=vD[0][:])
                    continue
                if K_STAGE == 2:
                    if mt == 0 and tc4 == 0:
                        nc.gpsimd.dma_start(out=out[0:32, :],
                                            in_=stqT[:, 0:2048])
                        nc.gpsimd.dma_start(out=out[32:64, :],
                                            in_=stqT[:, 2048:4096])
                        for r in range(4):
                            nc.gpsimd.dma_start(
                                out=out[64 + r * 32:96 + r * 32, :],
                                in_=bdkT[:, r * 2048:(r + 1) * 2048])
                        nc.gpsimd.dma_start(out=out[192:320, 0:2048],
                                            in_=bdvT[:, 0:2048])
                        nc.gpsimd.dma_start(out=out[320:448, 0:128],
                                            in_=bdvT[:, 2048:2176])
                        nc.gpsimd.dma_start(out=out[448:576, 0:1024],
                                            in_=qm[:])
                        nc.gpsimd.dma_start(out=out[576:704, 0:1024],
                                            in_=km[:])
                        nc.gpsimd.dma_start(out=out[704:832, 0:512],
                                            in_=vD[0][:])
                    continue
                # ---- attention: E = bdkT^T stqT; A = exp(E); AV + Z ----
                hdw = p_hd.tile([64, 16, 128], BF16, tag="hdw",
                                name=f"hdw{mt}_{tc4}")  # [i][h][t]
                for batch in range(8):  # 16 tokens
                    bt = tc4 * 8 + batch
                    psE = ps_E.tile([128, 512], F32, tag="psE",
                                    name=f"psE{mt}_{bt}")
                    for g8 in range(8):
                        g = batch * 8 + g8      # token-pair index in tc4
                        nc.tensor.matmul(
                            out=psE[:, g8 * 64:(g8 + 1) * 64],
                            lhsT=bdkT[:, g * 128:(g + 1) * 128],
                            rhs=stqT[:, g * 64:(g + 1) * 64],
                            start=True, stop=True)
                    expE = p_exp.tile([128, 512], BF16, tag="expE",
                                      name=f"expE{mt}_{bt}")
                    nc.scalar.activation(out=expE[:], in_=psE[:], func=Exp)
                    ps2 = ps_2.tile([64, 272], F32, tag="ps2",
                                    name=f"ps2{mt}_{bt}")
                    bdvT_g = bdvT[:].rearrange("p (c g) -> p g c", g=64)
                    for g8 in range(8):
                        g = batch * 8 + g8
                        nc.tensor.matmul(
                            out=ps2[:, g8 * 34:(g8 + 1) * 34],
                            lhsT=expE[:, g8 * 64:(g8 + 1) * 64],
                            rhs=bdvT_g[:, g, :],
                            start=True, stop=True)
                    ps2v = ps2[:].rearrange("p (g c) -> p g c", c=34)
                    nc.vector.reciprocal(
                        rcp64[:, tc4 * 128:(tc4 + 1) * 128].rearrange(
                            "p (b g) -> p g b", b=2)[:, batch * 8:(batch + 1) * 8],
                        ps2v[:, :, 32:34])
                    # hdw[i, h, t=(b,g)] <- ps2 cols (g,(b,h))
                    nc.vector.tensor_copy(
                        hdw[:].rearrange(
                            "p h (b g) -> p h b g", b=2)[:, :, :, batch * 8:(batch + 1) * 8],
                        ps2v[:, :, 0:32].rearrange("p g (b h) -> p h b g",
                                                   h=16))

                # concatT[kc][(hb,i), tc4-slice] <- hdw[i, 2kc+hb, t]
                for kc in range(8):
                    for hb in range(2):
                        nc.sync.dma_start(
                            out=concatT[kc][hb * 64:(hb + 1) * 64,
                                            tc4 * 128:(tc4 + 1) * 128],
                            in_=hdw[:, 2 * kc + hb, :])

            if K_STAGE == 2:
                continue
            if K_STAGE == 2:
                continue
            # ---- normalize + output projection ----
            rcp128 = p_rcp.tile([128, MT], F32, tag="rcp128",
                                name=f"rcp128_{mt}")
            nc.vector.tensor_copy(rcp128[0:64, :], rcp64[:])
            nc.sync.dma_start(out=rcp128[64:128, :], in_=rcp64[:])
            ccb = []
            for kc in range(8):
                cb = p_cc.tile([128, MT], BF16, tag="ccb",
                               name=f"ccb{mt}_{kc}")
                nc.vector.tensor_tensor(out=cb[:], in0=concatT[kc][:],
                                        in1=rcp128[:], op=Mult)
                ccb.append(cb)
            for oc in range(8):
                ps = ps_o.tile([128, 512], F32, tag="pso",
                                name=f"pspo{mt}_{oc}")
                for kc in range(8):
                    nc.tensor.matmul(out=ps[:],
                                     lhsT=wo_sb[kc][:, oc * 128:(oc + 1) * 128],
                                     rhs=ccb[kc][:],
                                     start=(kc == 0), stop=(kc == 7))
                ot = p_outT.tile([128, MT], F32, tag="outT",
                                 name=f"outT{mt}_{oc}")
                nc.scalar.activation(out=ot[:], in_=ps[:], func=Copy)
                nc.sync.dma_start(out=out[oc * 128:(oc + 1) * 128, t0:t0 + MT],
                                  in_=ot[:])
    nc.compile()
    return nc


_NC_CACHE = None


def _get_nc():
    global _NC_CACHE
    if _NC_CACHE is None:
        _NC_CACHE = build_nc()
    return _NC_CACHE


def _host_prep(queries, keys, values, mask, Wq, Wk, Wv, Wo):
    """Build the 8 per-core input maps."""
    import ml_dtypes
    bf = lambda a: np.ascontiguousarray(a).astype(ml_dtypes.bfloat16)
    fq = bf(queries.reshape(N * T, D).T)  # [D, 16384] bf16
    fk = bf(keys.reshape(N * T, D).T)
    fv = bf(values.reshape(N * T, D).T)
    fm = mask.reshape(N * T).astype(np.float32) / 32.0

    wq_h = bf(Wq.T)
    wk_h = bf(Wk.T)
    wv_h = bf(Wv.T)  # natural (h,j) column order: xbar windows = head pairs
    kpp = np.arange(D)
    perm = (kpp % 64) * 16 + (kpp // 64)  # k''=h*64+i -> source row i*16+h
    wo_h = bf(Wo.T[perm])

    in_maps = []
    for c in range(N_CORES):
        s = slice(c * TOK, (c + 1) * TOK)
        in_maps.append({
            "xq": np.ascontiguousarray(fq[:, s]),
            "xk": np.ascontiguousarray(fk[:, s]),
            "xv": np.ascontiguousarray(fv[:, s]),
            "wq": wq_h, "wk": wk_h, "wv": wv_h, "wo": wo_h,
            "m32": np.ascontiguousarray(fm[s].reshape(TOK // 128, 128).T),
        })
    return in_maps


def kernel(queries, keys, values, mask, Wq, Wk, Wv, Wo, _trace=False, _tmpdir=None):
    queries = np.asarray(queries, dtype=np.float32)
    keys = np.asarray(keys, dtype=np.float32)
    values = np.asarray(values, dtype=np.float32)
    mask = np.asarray(mask)
    in_maps = _host_prep(queries, keys, values, mask,
                         np.asarray(Wq, np.float32), np.asarray(Wk, np.float32),
                         np.asarray(Wv, np.float32), np.asarray(Wo, np.float32))
    nc = _get_nc()
    res = run_bass_kernel_spmd(nc, in_maps, core_ids=list(range(N_CORES)),
                               trace=_trace, tmpdir=_tmpdir)
    outs = []
    for c in range(N_CORES):
        outs.append(np.asarray(res.results[c]["out"]).T)  # [TOK, D]
    full = np.concatenate(outs, axis=0).reshape(N, T, D)
    kernel.last_exec_time_ns = res.exec_time_ns
    return full
